# revision 3
# baseline (speedup 1.0000x reference)
"""Trainium2 Bass kernel for nn_CausalSelfAttention_17188459119385.

Sharding: 8 cores = batch (2) x KV-head groups (4).  Core c handles batch
c//4 and KV head c%4 (with its 4 grouped query heads).  Each core computes
a partial output y_part = attn_out @ w_o[rows of its heads]; the host sums
the 4 partials per batch and adds b_o.

Device dataflow (all matmul operands bf16, fp32 PSUM accumulation):
  - x[b] is cast to bf16 on host; DMA-transpose loads x^T [C, T] into SBUF.
  - Q^T = w_q^T x^T, K^T = w_k^T x^T (feature-on-partition layout),
    V natural [T, D] with a ones-column appended (so the PV matmul also
    produces the softmax row-sums for free).
  - Scores S^T[s, t] = (K^T)^T Q^T per head; softmax skips the max
    subtraction (logits are bounded ~3.4) so exp is a single ACT pass with
    the 1/sqrt(D) scale folded in; causal masking via memset + one
    triangular mask multiply on diagonal 128-blocks.
  - O~^T accumulates over s-blocks in PSUM; row 64 is the rowsum.  The
    reciprocal rowsum is broadcast across partitions with a tiny ones
    matmul and multiplied in during the PSUM->SBUF copy.
  - y^T = w_o^T O^T streams out per 128-row chunk, fp32.
"""

import sys

if "/opt/trn_rl_repo" not in sys.path:
    sys.path.insert(0, "/opt/trn_rl_repo")

import numpy as np
import ml_dtypes

B, T, C = 2, 2048, 1024
NKV, G, D = 4, 4, 64          # kv heads, q-heads per kv head, head dim
QD = G * D                    # 256: q-feature width per core
P = 128
TCH = 512                     # t-chunk (matmul moving width)
NT = T // TCH                 # 4
NCC = C // P                  # 8 contraction chunks
NS = T // P                   # 16 s-blocks
BF16 = ml_dtypes.bfloat16

_CACHE = {}


def _build_nc():
    import concourse.mybir as mybir
    from concourse import bacc
    from concourse.tile import TileContext

    dt = mybir.dt
    AF = mybir.ActivationFunctionType

    nc = bacc.Bacc("TRN2", target_bir_lowering=False, debug=False)

    xb = nc.dram_tensor("xb", [T, C], dt.bfloat16, kind="ExternalInput")
    wq = nc.dram_tensor("wq", [C, QD], dt.bfloat16, kind="ExternalInput")
    wk = nc.dram_tensor("wk", [C, 2 * D], dt.bfloat16, kind="ExternalInput")
    wv = nc.dram_tensor("wv", [C, D], dt.bfloat16, kind="ExternalInput")
    wo = nc.dram_tensor("wo", [QD, C], dt.bfloat16, kind="ExternalInput")
    bq = nc.dram_tensor("bq", [P, 2], dt.float32, kind="ExternalInput")
    bk = nc.dram_tensor("bk", [2 * D, 1], dt.float32, kind="ExternalInput")
    bvr = nc.dram_tensor("bvr", [P, D], dt.float32, kind="ExternalInput")
    msk = nc.dram_tensor("msk", [P, P], dt.bfloat16, kind="ExternalInput")
    ones = nc.dram_tensor("ones", [1, D], dt.bfloat16, kind="ExternalInput")
    yt = nc.dram_tensor("yt", [C, T], dt.float32, kind="ExternalOutput")

    with TileContext(nc) as tc:
        with (
            tc.tile_pool(name="const", bufs=1) as cpool,
            tc.tile_pool(name="xt", bufs=NCC) as xtpool,
            tc.tile_pool(name="qt", bufs=2) as qtpool,
            tc.tile_pool(name="kt", bufs=1) as ktpool,
            tc.tile_pool(name="v", bufs=1) as vpool,
            tc.tile_pool(name="ot", bufs=2) as otpool,
            tc.tile_pool(name="p", bufs=6) as ppool,
            tc.tile_pool(name="r", bufs=8) as rpool,
            tc.tile_pool(name="rbs", bufs=4) as rbspool,
            tc.tile_pool(name="y", bufs=3) as ypool,
            tc.tile_pool(name="otmp", bufs=4) as otmp,
            tc.tile_pool(name="mmps", bufs=2, space="PSUM") as mmps,
            tc.tile_pool(name="sps", bufs=2, space="PSUM") as sps,
            tc.tile_pool(name="ops", bufs=2, space="PSUM") as ops_,
            tc.tile_pool(name="rbps", bufs=2, space="PSUM") as rbps,
        ):
            # ---- constants ----
            wq_sb = cpool.tile([P, NCC, QD], dt.bfloat16, tag="wq")
            nc.sync.dma_start(wq_sb[:], wq.ap().rearrange("(a p) d -> p a d", p=P))
            wk_sb = cpool.tile([P, NCC, 2 * D], dt.bfloat16, tag="wk")
            nc.sync.dma_start(wk_sb[:], wk.ap().rearrange("(a p) d -> p a d", p=P))
            wv_sb = cpool.tile([P, NCC, D], dt.bfloat16, tag="wv")
            nc.sync.dma_start(wv_sb[:], wv.ap().rearrange("(a p) d -> p a d", p=P))
            wo_sb = cpool.tile([P, 2, C], dt.bfloat16, tag="wo")
            nc.sync.dma_start(wo_sb[:], wo.ap().rearrange("(a p) e -> p a e", p=P))
            bq_sb = cpool.tile([P, 2], dt.float32, tag="bq")
            nc.sync.dma_start(bq_sb[:], bq[:])
            bk_sb = cpool.tile([2 * D, 1], dt.float32, tag="bk")
            nc.sync.dma_start(bk_sb[:], bk[:])
            bvr_sb = cpool.tile([P, D], dt.float32, tag="bvr")
            nc.sync.dma_start(bvr_sb[:], bvr[:])
            msk_sb = cpool.tile([P, P], dt.bfloat16, tag="msk")
            nc.sync.dma_start(msk_sb[:], msk[:])
            ones_sb = cpool.tile([1, D], dt.bfloat16, tag="ones")
            nc.sync.dma_start(ones_sb[:], ones[:])

            # ---- x^T via DMA transpose ----
            xt = []
            for a in range(NCC):
                xa = xtpool.tile([P, T], dt.bfloat16, tag="xt", name=f"xt{a}")
                nc.sync.dma_start(out=xa[:], in_=xb[:, a * P:(a + 1) * P],
                                  transpose=True)
                xt.append(xa)

            # ---- projections ----
            Qt = [qtpool.tile([P, T], dt.bfloat16, tag="qt", name=f"qt{i}")
                  for i in range(2)]
            for qc in range(2):
                for ti in range(NT):
                    ps = mmps.tile([P, TCH], dt.float32, tag="mm")
                    for a in range(NCC):
                        nc.tensor.matmul(
                            ps[:],
                            wq_sb[:, a, qc * P:(qc + 1) * P],
                            xt[a][:, ti * TCH:(ti + 1) * TCH],
                            start=(a == 0), stop=(a == NCC - 1),
                        )
                    nc.vector.tensor_scalar_add(
                        Qt[qc][:, ti * TCH:(ti + 1) * TCH], ps[:],
                        bq_sb[:, qc:qc + 1],
                    )

            Kt = ktpool.tile([P, T], dt.bfloat16, tag="kt")
            for ti in range(NT):
                ps = mmps.tile([P, TCH], dt.float32, tag="mm")
                for a in range(NCC):
                    nc.tensor.matmul(
                        ps[:], wk_sb[:, a, :],
                        xt[a][:, ti * TCH:(ti + 1) * TCH],
                        start=(a == 0), stop=(a == NCC - 1),
                    )
                nc.vector.tensor_scalar_add(
                    Kt[:, ti * TCH:(ti + 1) * TCH], ps[:], bk_sb[:, 0:1])

            Vb = vpool.tile([P, NS, D + 1], dt.bfloat16, tag="v")
            nc.gpsimd.memset(Vb[:], 1.0)
            for si in range(NS):
                ps = mmps.tile([P, D], dt.float32, tag="mm")
                for a in range(NCC):
                    nc.tensor.matmul(
                        ps[:], xt[a][:, si * P:(si + 1) * P], wv_sb[:, a, :],
                        start=(a == 0), stop=(a == NCC - 1),
                    )
                nc.vector.tensor_add(Vb[:, si, 0:D], ps[:], bvr_sb[:])

            # ---- attention + output projection ----
            Ot = [otpool.tile([P, T], dt.bfloat16, tag="ot", name=f"ot{i}")
                  for i in range(2)]
            for ti in range(NT):
                t0 = ti * TCH
                for h in range(G):
                    qc, qr = divmod(h, 2)
                    q_ap = Qt[qc][qr * D:(qr + 1) * D, t0:t0 + TCH]
                    o_ps = ops_.tile([D + 1, TCH], dt.float32, tag="o")
                    nsb = (t0 + TCH) // P
                    for sb in range(nsb):
                        s0 = sb * P
                        s_ps = sps.tile([P, TCH], dt.float32, tag="s")
                        nc.tensor.matmul(s_ps[:],
                                         Kt[qr * D:(qr + 1) * D, s0:s0 + P],
                                         q_ap,
                                         start=True, stop=True)
                        pt = ppool.tile([P, TCH], dt.bfloat16, tag="p")
                        nc.scalar.activation(pt[:], s_ps[:], AF.Exp, scale=0.125)
                        if s0 >= t0:
                            j0 = s0 - t0
                            if j0 > 0:
                                nc.gpsimd.memset(pt[:, 0:j0], 0.0)
                            nc.vector.tensor_mul(
                                pt[:, j0:j0 + P], pt[:, j0:j0 + P], msk_sb[:])
                        nc.tensor.matmul(o_ps[:], Vb[:, sb, :], pt[:],
                                         start=(sb == 0), stop=(sb == nsb - 1))
                    rr = rpool.tile([1, TCH], dt.bfloat16, tag="rr")
                    with nc.allow_low_precision(reason="softmax recip bf16"):
                        nc.vector.reciprocal(rr[:], o_ps[D:D + 1, :])
                    rb_ps = rbps.tile([D, TCH], dt.float32, tag="rb")
                    nc.tensor.matmul(rb_ps[:], ones_sb[:], rr[:],
                                     start=True, stop=True)
                    rb_sb = rbspool.tile([D, TCH], dt.float32, tag="rbs")
                    nc.vector.tensor_copy(rb_sb[:], rb_ps[:])
                    if qr == 0:
                        nc.vector.tensor_mul(
                            Ot[qc][0:D, t0:t0 + TCH], o_ps[0:D, :], rb_sb[:])
                    else:
                        ott = otmp.tile([D, TCH], dt.bfloat16, tag="ott",
                                        name="ott")
                        nc.vector.tensor_mul(ott[:], o_ps[0:D, :], rb_sb[:])
                        nc.sync.dma_start(
                            Ot[qc][D:2 * D, t0:t0 + TCH], ott[:])
                for ec in range(8):
                    y_ps = mmps.tile([P, TCH], dt.float32, tag="mm")
                    for dc in range(2):
                        nc.tensor.matmul(
                            y_ps[:], wo_sb[:, dc, ec * P:(ec + 1) * P],
                            Ot[dc][:, t0:t0 + TCH],
                            start=(dc == 0), stop=(dc == 1),
                        )
                    y_sb = ypool.tile([P, TCH], dt.float32, tag="y")
                    nc.vector.tensor_copy(y_sb[:], y_ps[:])
                    nc.sync.dma_start(
                        yt[ec * P:(ec + 1) * P, t0:t0 + TCH], y_sb[:])

    nc.compile()
    return nc


def get_nc():
    if "nc" not in _CACHE:
        _CACHE["nc"] = _build_nc()
    return _CACHE["nc"]


def make_in_maps(x, w_q, b_q, w_k, b_k, w_v, b_v, w_o, b_o):
    """Host-side sharding: per-core input maps for cores 0..7."""
    tri = np.triu(np.ones((P, P), np.float32)).astype(BF16)  # keep s<=t
    ones = np.ones((1, D), BF16)
    in_maps = []
    for c in range(8):
        b, kv = divmod(c, NKV)
        q0 = kv * QD
        in_maps.append({
            "xb": np.ascontiguousarray(x[b]).astype(BF16),
            "wq": np.ascontiguousarray(w_q[:, q0:q0 + QD]).astype(BF16),
            "wk": np.ascontiguousarray(
                np.concatenate([w_k[:, kv * D:(kv + 1) * D]] * 2, axis=1)
            ).astype(BF16),
            "wv": np.ascontiguousarray(w_v[:, kv * D:(kv + 1) * D]).astype(BF16),
            "wo": np.ascontiguousarray(w_o[q0:q0 + QD, :]).astype(BF16),
            "bq": np.ascontiguousarray(
                b_q[q0:q0 + QD].astype(np.float32).reshape(2, P).T),
            "bk": np.tile(
                b_k[kv * D:(kv + 1) * D].astype(np.float32), 2).reshape(2 * D, 1),
            "bvr": np.tile(
                b_v[kv * D:(kv + 1) * D].astype(np.float32)[None, :], (P, 1)),
            "msk": tri,
            "ones": ones,
        })
    return in_maps


def kernel(x, w_q, b_q, w_k, b_k, w_v, b_v, w_o, b_o):
    from concourse.bass_utils import run_bass_kernel_spmd

    x = np.asarray(x)
    nc = get_nc()
    in_maps = make_in_maps(x, np.asarray(w_q), np.asarray(b_q),
                           np.asarray(w_k), np.asarray(b_k),
                           np.asarray(w_v), np.asarray(b_v),
                           np.asarray(w_o), np.asarray(b_o))
    res = run_bass_kernel_spmd(nc, in_maps, list(range(8)))
    out = np.zeros((B, T, C), np.float32)
    for c in range(8):
        out[c // NKV] += res.results[c]["yt"].T
    out += np.asarray(b_o).astype(np.float32)[None, None, :]
    return out


# revision 8
# speedup vs baseline: 1.3263x; 1.3263x over previous
"""Trainium2 Bass kernel for nn_CausalSelfAttention_17188459119385.

Sharding: 8 cores = batch (2) x KV-head groups (4).  Core c handles batch
c//4 and KV head c%4 (with its 4 grouped query heads).  Each core computes
a partial output y_part = attn_out @ w_o[rows of its heads]; the host sums
the 4 partials per batch and adds b_o.

Device dataflow (all matmul operands bf16, fp32 PSUM accumulation):
  - x[b] is cast to bf16 on host; DMA-transpose loads x^T [C, T] into SBUF.
  - Q^T = w_q^T x^T, K^T = w_k^T x^T (feature-on-partition layout),
    V natural [T, D] with a ones-column appended (so the PV matmul also
    produces the softmax row-sums for free).
  - Scores S^T[s, t] = (K^T)^T Q^T per head; softmax skips the max
    subtraction (logits are bounded ~3.4) so exp is a single ACT pass with
    the 1/sqrt(D) scale folded in; causal masking via memset + one
    triangular mask multiply on diagonal 128-blocks.
  - O~^T accumulates over s-blocks in PSUM; row 64 is the rowsum.  The
    reciprocal rowsum is broadcast across partitions with a tiny ones
    matmul and multiplied in during the PSUM->SBUF copy.
  - y^T = w_o^T O^T streams out per 128-row chunk, fp32.
"""

import sys

if "/opt/trn_rl_repo" not in sys.path:
    sys.path.insert(0, "/opt/trn_rl_repo")

import numpy as np
import ml_dtypes

B, T, C = 2, 2048, 1024
NKV, G, D = 4, 4, 64          # kv heads, q-heads per kv head, head dim
QD = G * D                    # 256: q-feature width per core
P = 128
TCH = 512                     # t-chunk (matmul moving width)
NT = T // TCH                 # 4
NCC = C // P                  # 8 contraction chunks
NS = T // P                   # 16 s-blocks
BF16 = ml_dtypes.bfloat16

_CACHE = {}


def _build_nc():
    import concourse.mybir as mybir
    from concourse import bacc
    from concourse.tile import TileContext

    dt = mybir.dt
    AF = mybir.ActivationFunctionType

    nc = bacc.Bacc("TRN2", target_bir_lowering=False, debug=False)

    xb = nc.dram_tensor("xb", [T, C], dt.bfloat16, kind="ExternalInput")
    wq = nc.dram_tensor("wq", [C, QD], dt.bfloat16, kind="ExternalInput")
    wk = nc.dram_tensor("wk", [C, 2 * D], dt.bfloat16, kind="ExternalInput")
    wv = nc.dram_tensor("wv", [C, D], dt.bfloat16, kind="ExternalInput")
    wo = nc.dram_tensor("wo", [QD, C], dt.bfloat16, kind="ExternalInput")
    bq = nc.dram_tensor("bq", [P, 2], dt.float32, kind="ExternalInput")
    bk = nc.dram_tensor("bk", [2 * D, 1], dt.float32, kind="ExternalInput")
    bvr = nc.dram_tensor("bvr", [P, D], dt.float32, kind="ExternalInput")
    msk = nc.dram_tensor("msk", [P, P], dt.bfloat16, kind="ExternalInput")
    ones = nc.dram_tensor("ones", [1, D], dt.bfloat16, kind="ExternalInput")
    yt = nc.dram_tensor("yt", [C, T], dt.float32, kind="ExternalOutput")

    with TileContext(nc) as tc:
        with (
            tc.tile_pool(name="const", bufs=1) as cpool,
            tc.tile_pool(name="xt", bufs=NCC * NT) as xtpool,
            tc.tile_pool(name="qt", bufs=2) as qtpool,
            tc.tile_pool(name="kt", bufs=1) as ktpool,
            tc.tile_pool(name="v", bufs=1) as vpool,
            tc.tile_pool(name="ot", bufs=2) as otpool,
            tc.tile_pool(name="p", bufs=8) as ppool,
            tc.tile_pool(name="r", bufs=8) as rpool,
            tc.tile_pool(name="rbs", bufs=4) as rbspool,
            tc.tile_pool(name="y", bufs=3) as ypool,
            tc.tile_pool(name="otmp", bufs=4) as otmp,
            tc.tile_pool(name="mmps", bufs=2, space="PSUM") as mmps,
            tc.tile_pool(name="sps", bufs=3, space="PSUM") as sps,
            tc.tile_pool(name="ops", bufs=2, space="PSUM") as ops_,
            tc.tile_pool(name="rbps", bufs=1, space="PSUM") as rbps,
        ):
            # ---- constants ----
            wq_sb = cpool.tile([P, NCC, QD], dt.bfloat16, tag="wq")
            nc.sync.dma_start(wq_sb[:], wq.ap().rearrange("(a p) d -> p a d", p=P))
            wk_sb = cpool.tile([P, NCC, 2 * D], dt.bfloat16, tag="wk")
            nc.sync.dma_start(wk_sb[:], wk.ap().rearrange("(a p) d -> p a d", p=P))
            wv_sb = cpool.tile([P, NCC, D], dt.bfloat16, tag="wv")
            nc.sync.dma_start(wv_sb[:], wv.ap().rearrange("(a p) d -> p a d", p=P))
            wo_sb = cpool.tile([P, 2, C], dt.bfloat16, tag="wo")
            nc.sync.dma_start(wo_sb[:], wo.ap().rearrange("(a p) e -> p a e", p=P))
            bq_sb = cpool.tile([P, 2], dt.float32, tag="bq")
            nc.sync.dma_start(bq_sb[:], bq[:])
            bk_sb = cpool.tile([2 * D, 1], dt.float32, tag="bk")
            nc.sync.dma_start(bk_sb[:], bk[:])
            bvr_sb = cpool.tile([P, D], dt.float32, tag="bvr")
            nc.sync.dma_start(bvr_sb[:], bvr[:])
            msk_sb = cpool.tile([P, P], dt.bfloat16, tag="msk")
            nc.sync.dma_start(msk_sb[:], msk[:])
            ones_sb = cpool.tile([1, D], dt.bfloat16, tag="ones")
            nc.sync.dma_start(ones_sb[:], ones[:])

            # ---- x^T via DMA transpose (contiguous [P, TCH] dests) ----
            xt = []
            for a in range(NCC):
                row = []
                for q in range(NT):
                    xa = xtpool.tile([P, TCH], dt.bfloat16, tag="xt",
                                     name=f"xt{a}_{q}")
                    nc.sync.dma_start(
                        out=xa[:],
                        in_=xb[q * TCH:(q + 1) * TCH, a * P:(a + 1) * P],
                        transpose=True)
                    row.append(xa)
                xt.append(row)

            # ---- projections ----
            Qt = [qtpool.tile([P, T], dt.bfloat16, tag="qt", name=f"qt{i}")
                  for i in range(2)]
            for qc in range(2):
                for ti in range(NT):
                    ps = mmps.tile([P, TCH], dt.float32, tag="mm")
                    for a in range(NCC):
                        nc.tensor.matmul(
                            ps[:],
                            wq_sb[:, a, qc * P:(qc + 1) * P],
                            xt[a][ti][:],
                            start=(a == 0), stop=(a == NCC - 1),
                        )
                    nc.vector.tensor_scalar_add(
                        Qt[qc][:, ti * TCH:(ti + 1) * TCH], ps[:],
                        bq_sb[:, qc:qc + 1],
                    )

            Kt = ktpool.tile([P, T], dt.bfloat16, tag="kt")
            for ti in range(NT):
                ps = mmps.tile([P, TCH], dt.float32, tag="mm")
                for a in range(NCC):
                    nc.tensor.matmul(
                        ps[:], wk_sb[:, a, :],
                        xt[a][ti][:],
                        start=(a == 0), stop=(a == NCC - 1),
                    )
                nc.vector.tensor_scalar_add(
                    Kt[:, ti * TCH:(ti + 1) * TCH], ps[:], bk_sb[:, 0:1])

            Vb = vpool.tile([P, NS, D + 1], dt.bfloat16, tag="v")
            nc.gpsimd.memset(Vb[:], 1.0)
            for si in range(NS):
                ps = mmps.tile([P, D], dt.float32, tag="mm")
                for a in range(NCC):
                    nc.tensor.matmul(
                        ps[:],
                        xt[a][si // 4][:, (si % 4) * P:(si % 4 + 1) * P],
                        wv_sb[:, a, :],
                        start=(a == 0), stop=(a == NCC - 1),
                    )
                nc.vector.tensor_add(Vb[:, si, 0:D], ps[:], bvr_sb[:])

            # ---- attention + output projection ----
            Ot = [otpool.tile([P, T], dt.bfloat16, tag="ot", name=f"ot{i}")
                  for i in range(2)]
            for ti in range(NT):
                t0 = ti * TCH
                for h in range(G):
                    qc, qr = divmod(h, 2)
                    q_ap = Qt[qc][qr * D:(qr + 1) * D, t0:t0 + TCH]
                    o_ps = ops_.tile([D + 1, TCH], dt.float32, tag="o")
                    nsb = (t0 + TCH) // P
                    for sb in range(nsb):
                        s0 = sb * P
                        s_ps = sps.tile([P, TCH], dt.float32, tag="s")
                        nc.tensor.matmul(s_ps[:],
                                         Kt[qr * D:(qr + 1) * D, s0:s0 + P],
                                         q_ap,
                                         start=True, stop=True)
                        pt = ppool.tile([P, TCH], dt.bfloat16, tag="p")
                        if s0 >= t0:
                            j0 = s0 - t0
                            nc.scalar.activation(pt[:, j0:], s_ps[:, j0:],
                                                 AF.Exp, scale=0.125)
                            if j0 > 0:
                                nc.gpsimd.memset(pt[:, 0:j0], 0.0)
                            nc.vector.tensor_mul(
                                pt[:, j0:j0 + P], pt[:, j0:j0 + P], msk_sb[:])
                        else:
                            nc.scalar.activation(pt[:], s_ps[:], AF.Exp,
                                                 scale=0.125)
                        nc.tensor.matmul(o_ps[:], Vb[:, sb, :], pt[:],
                                         start=(sb == 0), stop=(sb == nsb - 1))
                    rs = rpool.tile([1, TCH], dt.float32, tag="rs")
                    nc.vector.tensor_copy(rs[:], o_ps[D:D + 1, :])
                    rr = rpool.tile([1, TCH], dt.float32, tag="rr")
                    nc.vector.reciprocal_approx_fast(rr[:], rs[:])
                    rrb = rpool.tile([1, TCH], dt.bfloat16, tag="rrb")
                    nc.vector.tensor_copy(rrb[:], rr[:])
                    rb_ps = rbps.tile([D, TCH], dt.float32, tag="rb")
                    nc.tensor.matmul(rb_ps[:], ones_sb[:], rrb[:],
                                     start=True, stop=True)
                    rb_sb = rbspool.tile([D, TCH], dt.float32, tag="rbs")
                    nc.vector.tensor_copy(rb_sb[:], rb_ps[:])
                    if qr == 0:
                        nc.vector.tensor_mul(
                            Ot[qc][0:D, t0:t0 + TCH], o_ps[0:D, :], rb_sb[:])
                    else:
                        ott = otmp.tile([D, TCH], dt.bfloat16, tag="ott",
                                        name="ott")
                        nc.vector.tensor_mul(ott[:], o_ps[0:D, :], rb_sb[:])
                        nc.sync.dma_start(
                            Ot[qc][D:2 * D, t0:t0 + TCH], ott[:])
                for ec in range(8):
                    y_ps = mmps.tile([P, TCH], dt.float32, tag="mm")
                    for dc in range(2):
                        nc.tensor.matmul(
                            y_ps[:], wo_sb[:, dc, ec * P:(ec + 1) * P],
                            Ot[dc][:, t0:t0 + TCH],
                            start=(dc == 0), stop=(dc == 1),
                        )
                    y_sb = ypool.tile([P, TCH], dt.float32, tag="y")
                    nc.vector.tensor_copy(y_sb[:], y_ps[:])
                    nc.sync.dma_start(
                        yt[ec * P:(ec + 1) * P, t0:t0 + TCH], y_sb[:])

    nc.compile()
    return nc


def get_nc():
    if "nc" not in _CACHE:
        _CACHE["nc"] = _build_nc()
    return _CACHE["nc"]


def make_in_maps(x, w_q, b_q, w_k, b_k, w_v, b_v, w_o, b_o):
    """Host-side sharding: per-core input maps for cores 0..7."""
    tri = np.triu(np.ones((P, P), np.float32)).astype(BF16)  # keep s<=t
    ones = np.ones((1, D), BF16)
    in_maps = []
    for c in range(8):
        b, kv = divmod(c, NKV)
        q0 = kv * QD
        in_maps.append({
            "xb": np.ascontiguousarray(x[b]).astype(BF16),
            "wq": np.ascontiguousarray(w_q[:, q0:q0 + QD]).astype(BF16),
            "wk": np.ascontiguousarray(
                np.concatenate([w_k[:, kv * D:(kv + 1) * D]] * 2, axis=1)
            ).astype(BF16),
            "wv": np.ascontiguousarray(w_v[:, kv * D:(kv + 1) * D]).astype(BF16),
            "wo": np.ascontiguousarray(w_o[q0:q0 + QD, :]).astype(BF16),
            "bq": np.ascontiguousarray(
                b_q[q0:q0 + QD].astype(np.float32).reshape(2, P).T),
            "bk": np.tile(
                b_k[kv * D:(kv + 1) * D].astype(np.float32), 2).reshape(2 * D, 1),
            "bvr": np.tile(
                b_v[kv * D:(kv + 1) * D].astype(np.float32)[None, :], (P, 1)),
            "msk": tri,
            "ones": ones,
        })
    return in_maps


def kernel(x, w_q, b_q, w_k, b_k, w_v, b_v, w_o, b_o):
    from concourse.bass_utils import run_bass_kernel_spmd

    x = np.asarray(x)
    nc = get_nc()
    in_maps = make_in_maps(x, np.asarray(w_q), np.asarray(b_q),
                           np.asarray(w_k), np.asarray(b_k),
                           np.asarray(w_v), np.asarray(b_v),
                           np.asarray(w_o), np.asarray(b_o))
    res = run_bass_kernel_spmd(nc, in_maps, list(range(8)))
    out = np.zeros((B, T, C), np.float32)
    for c in range(8):
        out[c // NKV] += res.results[c]["yt"].T
    out += np.asarray(b_o).astype(np.float32)[None, None, :]
    return out
